# revision 1
# baseline (speedup 1.0000x reference)
"""nn_CostVolume kernel for 8 Trainium2 NeuronCores.

Sharding: query axis S is split into 4 chunks per batch; with B=2 that gives
8 independent shards, one per NeuronCore (core = b*4 + chunk). f2 cloud,
warped cloud (as KNN database) and all weights are replicated per core.

Stage 1 (cross-cloud attention) is computed per shard. Its output pi_feat1
is needed at arbitrary S positions by stage 2's self-KNN gather, so shards
are reassembled on host between the two stages, then stage 2 runs per shard.
Both stages are jit-compiled once per device and dispatched asynchronously
so all 8 cores run concurrently.
"""
import numpy as np
import jax
import jax.numpy as jnp

B, S, N = 2, 4096, 4096
NSAMPLE, NSAMPLE_Q = 16, 32
BN_EPS = 1e-5
NCHUNK = 4            # S-chunks per batch
CS = S // NCHUNK      # 1024 queries per core


def _shared_mlp(x, params):
    # x: (C, S, K) single-batch variant of the reference _shared_mlp
    for p in params:
        x = jnp.einsum('oi,isk->osk', p["W"], x)
        inv = jax.lax.rsqrt(p["var"] + BN_EPS)
        x = (x - p["mean"][:, None, None]) * (p["gamma"] * inv)[:, None, None] \
            + p["beta"][:, None, None]
        x = jax.nn.relu(x)
    return x


def _knn_idx(k, database, query):
    # database: (N,3), query: (Q,3) -> (Q,k)
    d = (jnp.sum(query * query, -1, keepdims=True)
         - 2.0 * jnp.einsum('sd,nd->sn', query, database)
         + jnp.sum(database * database, -1)[None, :])
    _, idx = jax.lax.top_k(-d, k)
    return idx


def _group(points, idx):
    # points: (C,N), idx: (Q,K) -> (C,Q,K)
    return jnp.transpose(points.T[idx], (2, 0, 1))


def _stage1(warped_xyz_c, warped_points_c, f2_xyz, f2_points,
            mlp1_params, xyz1_params, mlp2_params):
    # *_c are per-chunk (3, CS)/(64, CS); f2_* are full (3, N)/(C2, N)
    wt = warped_xyz_c.T                       # (CS,3)
    f2t = f2_xyz.T                            # (N,3)
    idx_q = _knn_idx(NSAMPLE_Q, f2t, wt)      # (CS,Kq)
    qi_xyz = _group(f2_xyz, idx_q)            # (3,CS,Kq)
    qi_pts = _group(f2_points, idx_q)         # (C2,CS,Kq)
    pi_xyz = warped_xyz_c[:, :, None]
    pi_pts = warped_points_c[:, :, None]
    xyz_diff = qi_xyz - pi_xyz
    euc = jnp.sqrt(jnp.sum(xyz_diff * xyz_diff, 0, keepdims=True) + 1e-20)
    pi_xyz_cat = jnp.concatenate(
        [jnp.broadcast_to(pi_xyz, qi_xyz.shape), qi_xyz, xyz_diff, euc], axis=0)
    pi_feat_cat = jnp.concatenate(
        [jnp.broadcast_to(pi_pts, qi_pts.shape), qi_pts], axis=0)
    pi_feat1 = _shared_mlp(jnp.concatenate([pi_xyz_cat, pi_feat_cat], axis=0),
                           mlp1_params)
    pi_xyz_enc = _shared_mlp(pi_xyz_cat, xyz1_params)
    wq = _shared_mlp(jnp.concatenate([pi_xyz_enc, pi_feat1], axis=0), mlp2_params)
    WQ = jax.nn.softmax(wq, axis=2)
    return jnp.sum(WQ * pi_feat1, axis=2)     # (mlp1[-1], CS)


def _stage2(warped_xyz_c, warped_points_c, warped_xyz_full, pi_feat1_full,
            xyz2_params, mlp3_params):
    wt_c = warped_xyz_c.T                     # (CS,3)
    wt_full = warped_xyz_full.T               # (S,3)
    idx = _knn_idx(NSAMPLE, wt_full, wt_c)    # (CS,K)
    pc_xyz_g = _group(warped_xyz_full, idx)   # (3,CS,K)
    pc_pts_g = _group(pi_feat1_full, idx)     # (C,CS,K)
    pi_xyz = warped_xyz_c[:, :, None]
    pi_pts = warped_points_c[:, :, None]
    pc_xyz_diff = pc_xyz_g - pi_xyz
    pc_euc = jnp.sqrt(jnp.sum(pc_xyz_diff * pc_xyz_diff, 0, keepdims=True) + 1e-20)
    pc_xyz_cat = jnp.concatenate(
        [jnp.broadcast_to(pi_xyz, pc_xyz_g.shape), pc_xyz_g, pc_xyz_diff, pc_euc],
        axis=0)
    pc_xyz_enc = _shared_mlp(pc_xyz_cat, xyz2_params)
    pc_concat = jnp.concatenate(
        [pc_xyz_enc,
         jnp.broadcast_to(pi_pts, (pi_pts.shape[0], CS, NSAMPLE)),
         pc_pts_g], axis=0)
    wp = _shared_mlp(pc_concat, mlp3_params)
    WP = jax.nn.softmax(wp, axis=2)
    return jnp.sum(WP * pc_pts_g, axis=2)     # (mlp2[-1], CS)


_stage1_jit = jax.jit(_stage1)
_stage2_jit = jax.jit(_stage2)


def kernel(warped_xyz, warped_points, f2_xyz, f2_points,
           mlp1_params, xyz1_params, xyz2_params, mlp2_params, mlp3_params):
    devices = jax.devices()[:8]

    def put_params(params, d):
        return [{k: jax.device_put(jnp.asarray(v), d) for k, v in p.items()}
                for p in params]

    # per-core shard descriptors: core i -> (batch, chunk)
    shards = [(i // NCHUNK, i % NCHUNK) for i in range(8)]

    # replicate weights / clouds per device
    dev_args = []
    for i, (b, c) in enumerate(shards):
        d = devices[i]
        sl = slice(c * CS, (c + 1) * CS)
        dev_args.append(dict(
            wxyz_c=jax.device_put(jnp.asarray(warped_xyz[b][:, sl]), d),
            wpts_c=jax.device_put(jnp.asarray(warped_points[b][:, sl]), d),
            f2_xyz=jax.device_put(jnp.asarray(f2_xyz[b]), d),
            f2_pts=jax.device_put(jnp.asarray(f2_points[b]), d),
            wxyz_full=jax.device_put(jnp.asarray(warped_xyz[b]), d),
            mlp1=put_params(mlp1_params, d),
            xyz1=put_params(xyz1_params, d),
            xyz2=put_params(xyz2_params, d),
            mlp2=put_params(mlp2_params, d),
            mlp3=put_params(mlp3_params, d),
        ))

    # ---- stage 1: dispatch all shards asynchronously ----
    s1 = [_stage1_jit(a["wxyz_c"], a["wpts_c"], a["f2_xyz"], a["f2_pts"],
                      a["mlp1"], a["xyz1"], a["mlp2"]) for a in dev_args]

    # reassemble pi_feat1 full per batch on host
    s1_np = [np.asarray(r) for r in s1]                    # (64, CS) each
    pi_feat1_full = [np.concatenate([s1_np[b * NCHUNK + c] for c in range(NCHUNK)],
                                    axis=1) for b in range(B)]  # (64, S)

    # ---- stage 2 ----
    outs = []
    for i, (b, c) in enumerate(shards):
        a = dev_args[i]
        pf = jax.device_put(jnp.asarray(pi_feat1_full[b]), devices[i])
        outs.append(_stage2_jit(a["wxyz_c"], a["wpts_c"], a["wxyz_full"], pf,
                                a["xyz2"], a["mlp3"]))

    out_np = [np.asarray(r) for r in outs]                 # (64, CS) each
    full = np.stack([np.concatenate([out_np[b * NCHUNK + c] for c in range(NCHUNK)],
                                    axis=1) for b in range(B)])
    return full.astype(np.float32)
